# revision 1
# baseline (speedup 1.0000x reference)
"""MoE (shared expert + 8 routed experts, top-2) on 8 Trainium2 NeuronCores.

Sharding: core c holds
  - shared-expert slice c: rows [c*1024, (c+1)*1024) of sw1/sw2 and the
    matching columns of sw3  -> partial (T, D) output, summed on host
  - routed expert c's weights (w12[c], w3[c]); host routes/gathers the
    tokens selected for expert c (capacity 1024 = the exact mean load),
    device computes unscaled expert outputs, host applies combine weights
    during the fp32 scatter-add; small per-expert overflows beyond the
    capacity are fixed up on host in fp32.

Device math is bf16 with fp32 PSUM accumulation; outputs are written
bf16 and promoted to fp32 during the host-side reduce.

v2 schedule: the routed expert runs FIRST (its 16.8MB of weights stream
at t=0 when the DMA queue is otherwise idle, with the contraction loop
ordered so matmuls start after ~0.5MB has landed); the shared-expert
weights prefetch behind routed compute so the phase transition has no
DMA bubble.  The v1 schedule ran shared-first and paid a ~30us stall +
HAM cold-clock window when the routed weights all loaded at the end.
"""

import sys

if "/opt/trn_rl_repo" not in sys.path:
    sys.path.insert(0, "/opt/trn_rl_repo")

from contextlib import ExitStack

import numpy as np
import ml_dtypes

import concourse.bass as bass
import concourse.tile as tile
from concourse import mybir, bacc
from concourse.bass_utils import run_bass_kernel_spmd

BF16 = mybir.dt.bfloat16
F32 = mybir.dt.float32
AF = mybir.ActivationFunctionType

# Problem shape (hardcoded per spec)
B, S, D = 2, 2048, 2048
T = B * S                  # 4096 tokens
E = 8                      # routed experts == n_cores
TOPK = 2
H_SHARED = 8192
HC = H_SHARED // 8         # shared-expert hidden slice per core
HR = 1024                  # routed expert hidden
NCORES = 8
NT = 512                   # token block (one PSUM bank at fp32)
P = 128
CH = 1024                  # shared-phase x chunk (2 sub-blocks)


def _build_program(C: int):
    """SPMD Bass program, routed capacity C (multiple of 128).

    Routed phase first, then shared; shared weights prefetch during the
    routed phase when C <= 1024 (SBUF budget), else after it.
    """
    nc = bacc.Bacc("TRN2", target_bir_lowering=False, debug=False)

    xT = nc.dram_tensor("xT", [D, T], BF16, kind="ExternalInput")
    sw1T = nc.dram_tensor("sw1T", [D, HC], BF16, kind="ExternalInput")
    sw2T = nc.dram_tensor("sw2T", [D, HC], BF16, kind="ExternalInput")
    sw3T = nc.dram_tensor("sw3T", [HC, D], BF16, kind="ExternalInput")
    # w12rT columns: [gate m0-3 | up m0-3 | gate m4-7 | up m4-7]
    w12rT = nc.dram_tensor("w12rT", [D, 2 * HR], BF16, kind="ExternalInput")
    w3T = nc.dram_tensor("w3T", [HR, D], BF16, kind="ExternalInput")
    xgT = nc.dram_tensor("xgT", [D, C], BF16, kind="ExternalInput")

    shared_outT = nc.dram_tensor("shared_outT", [D, T], BF16, kind="ExternalOutput")
    routed_outT = nc.dram_tensor("routed_outT", [D, C], BF16, kind="ExternalOutput")

    KD = D // P    # 16 contraction tiles over D
    KH = HC // P   # 8 contraction tiles over HC (== HR // P)
    H2 = HC // 2   # shared up-proj weight half-tile width

    # routed token blocks
    blk_w = [NT] * (C // NT)
    if C % NT:
        blk_w.append(C % NT)
    NBLK = len(blk_w)
    early_prefetch = C <= 1024

    with tile.TileContext(nc) as tc:
        with ExitStack() as ctx:
            # pools that live across both phases
            hpool = ctx.enter_context(tc.tile_pool(name="h_p", bufs=2))
            tpool = ctx.enter_context(tc.tile_pool(name="t_p", bufs=2))
            opool = ctx.enter_context(
                tc.tile_pool(name="o_p", bufs=4 if C <= 1024 else 2))
            # shared-phase first-half weights + chunk-0/sb-0 x: entered
            # before the routed pools (pool release is LIFO) so they
            # survive into the shared phase
            wsh1 = ctx.enter_context(tc.tile_pool(name="w_sh1", bufs=1))
            xA0 = ctx.enter_context(tc.tile_pool(name="x_a0", bufs=1))
            sw1_h1 = [wsh1.tile([P, H2], BF16, name=f"sw1_{k}_0",
                                tag=f"sw1_{k}_0") for k in range(KD)]
            sw2_h1 = [wsh1.tile([P, H2], BF16, name=f"sw2_{k}_0",
                                tag=f"sw2_{k}_0") for k in range(KD)]
            x00 = [xA0.tile([P, NT], BF16, name=f"xa_{k}", tag=f"xa_{k}")
                   for k in range(KD)]

            def emit_prefetch_a():
                for k in range(KD):
                    nc.sync.dma_start(sw1_h1[k][:], sw1T[k * P:(k + 1) * P, :H2])
                    nc.sync.dma_start(sw2_h1[k][:], sw2T[k * P:(k + 1) * P, :H2])
                for k in range(KD):
                    nc.sync.dma_start(x00[k][:], xT[k * P:(k + 1) * P, 0:NT])

            # HAM warmup: dummy matmuls on uninitialized SBUF (no deps, so
            # they issue right after the engine preamble) keep the PE busy
            # through its cold-clock window while the first weights stream
            # in; without them the first ~3.4us of real matmuls run at
            # half clock. Results land in a scratch PSUM bank, never read.
            wmp = ctx.enter_context(tc.tile_pool(name="wm_p", bufs=1))
            wwm = wmp.tile([P, P], BF16, name="wwm", tag="wwm")
            xwm = wmp.tile([P, NT // 2], BF16, name="xwm", tag="xwm")
            nc.vector.memset(wwm[:], 0)
            nc.gpsimd.memset(xwm[:], 0)

            with ExitStack() as ctx_r:
                # ---------------- Phase R: routed expert ----------------
                wr = ctx_r.enter_context(tc.tile_pool(name="w_r", bufs=1))
                xgp = ctx_r.enter_context(tc.tile_pool(name="xg_p", bufs=1))
                psR = ctx_r.enter_context(
                    tc.tile_pool(name="psR", bufs=1, space="PSUM"))

                for g in range(2):
                    pw = psR.tile([P, NT], F32, name="pb7", tag="pb7")
                    for k in range(8):
                        nc.tensor.matmul(pw[:, :NT // 2], wwm[:], xwm[:],
                                         start=(k == 0), stop=(k == 7))

                # full-width tiles, one DMA each: the sync engine issues
                # DMA instructions serially (~0.6us apiece, ring depth 1),
                # so fewer/bigger transfers beat many small ones
                w12_sb = [wr.tile([P, 2 * HR], BF16, name=f"w12_{k}",
                                  tag=f"w12_{k}") for k in range(KD)]
                w3_sb = [wr.tile([P, D], BF16, name=f"w3_{k}", tag=f"w3_{k}")
                         for k in range(KH)]
                xg_sb = [xgp.tile([P, C], BF16, name=f"xg_{k}", tag=f"xg_{k}")
                         for k in range(KD)]

                # DMA emission = execution order on the sync queue,
                # need-ordered: block-0 tokens + mg0 weight halves feed the
                # first matmul wave with exactly its bytes; then mg1
                # halves, then w3 (block-0 down-proj), then the remaining
                # tokens (block 1+)
                nt0 = min(NT, C)
                for k in range(KD):
                    if k < 2:
                        # first tiles split small across DMA rings: the
                        # first matmuls' dependencies land ~2us sooner
                        h0 = nt0 // 2
                        nc.sync.dma_start(xg_sb[k][:, 0:h0],
                                          xgT[k * P:(k + 1) * P, 0:h0])
                        nc.sync.dma_start(w12_sb[k][:, 0:NT],
                                          w12rT[k * P:(k + 1) * P, 0:NT])
                        nc.sync.dma_start(xg_sb[k][:, h0:nt0],
                                          xgT[k * P:(k + 1) * P, h0:nt0])
                        nc.sync.dma_start(w12_sb[k][:, NT:HR],
                                          w12rT[k * P:(k + 1) * P, NT:HR])
                    else:
                        nc.sync.dma_start(xg_sb[k][:, 0:nt0],
                                          xgT[k * P:(k + 1) * P, 0:nt0])
                        nc.sync.dma_start(w12_sb[k][:, 0:HR],
                                          w12rT[k * P:(k + 1) * P, 0:HR])
                for k in range(KD):
                    nc.sync.dma_start(w12_sb[k][:, HR:],
                                      w12rT[k * P:(k + 1) * P, HR:])
                for k in range(KH):
                    nc.sync.dma_start(w3_sb[k][:], w3T[k * P:(k + 1) * P, :])
                if C > nt0:
                    for k in range(KD):
                        nc.sync.dma_start(xg_sb[k][:, nt0:],
                                          xgT[k * P:(k + 1) * P, nt0:])

                if early_prefetch:
                    # shared-phase critical prefetch streams behind the
                    # routed loads
                    emit_prefetch_a()

                # routed compute
                off = 0
                for b, nt in enumerate(blk_w):
                    tok = slice(off, off + nt)
                    off += nt
                    hs = []
                    for mg in range(2):
                        pg = [psR.tile([P, NT], F32, name=f"pb{m}", tag=f"pb{m}")
                              for m in range(4)]
                        pu = [psR.tile([P, NT], F32, name=f"pb{4+m}", tag=f"pb{4+m}")
                              for m in range(4)]
                        # k emitted in quarters so the first matmuls only
                        # depend on the first few weight tiles
                        for kq in range(4):
                            ks = range(kq * 4, kq * 4 + 4)
                            for m in range(4):
                                # gate / up columns inside the mg half
                                c1 = slice(mg * HR + m * P, mg * HR + (m + 1) * P)
                                c2 = slice(mg * HR + NT + m * P,
                                           mg * HR + NT + (m + 1) * P)
                                for k in ks:
                                    nc.tensor.matmul(pg[m][:, :nt],
                                                     w12_sb[k][:, c1],
                                                     xg_sb[k][:, tok],
                                                     start=(k == 0), stop=(k == KD - 1))
                                for k in ks:
                                    nc.tensor.matmul(pu[m][:, :nt],
                                                     w12_sb[k][:, c2],
                                                     xg_sb[k][:, tok],
                                                     start=(k == 0), stop=(k == KD - 1))
                        for m in range(4):
                            sg = tpool.tile([P, NT], F32, name="sg", tag="sg")
                            nc.scalar.activation(sg[:, :nt], pg[m][:, :nt], AF.Silu)
                            h = hpool.tile([P, NT], BF16, name=f"h_{mg*4+m}",
                                           tag=f"h_{mg*4+m}")
                            nc.vector.tensor_mul(h[:, :nt], sg[:, :nt], pu[m][:, :nt])
                            hs.append(h)
                    for mo in range(KD):
                        po = psR.tile([P, NT], F32, name=f"pb{mo%8}", tag=f"pb{mo%8}")
                        for k in range(KH):
                            nc.tensor.matmul(po[:, :nt],
                                             w3_sb[k][:, mo * P:(mo + 1) * P],
                                             hs[k][:, :nt],
                                             start=(k == 0), stop=(k == KH - 1))
                        so = opool.tile([P, NT], BF16, name="so", tag="so")
                        nc.vector.tensor_copy(so[:, :nt], po[:, :nt])
                        # outputs ride the SWDGE so the sync queue keeps
                        # streaming weights
                        nc.gpsimd.dma_start(routed_outT[mo * P:(mo + 1) * P, tok],
                                            so[:, :nt])

            # ---------------- Phase S: shared expert ----------------
            wsh2 = ctx.enter_context(tc.tile_pool(name="w_sh2", bufs=1))
            wdn = ctx.enter_context(tc.tile_pool(name="w_dn", bufs=1))
            xpool = ctx.enter_context(tc.tile_pool(name="x_p", bufs=2))
            psA = ctx.enter_context(tc.tile_pool(name="psA", bufs=2, space="PSUM"))
            psB = ctx.enter_context(tc.tile_pool(name="psB", bufs=4, space="PSUM"))

            if not early_prefetch:
                emit_prefetch_a()

            # second weight halves, down-proj weights, chunk-0/sb-1 x:
            # these land in SBUF freed by the routed pools (WAR-ordered).
            # sw3 and x01 interleave so neither arrives at the wire.
            sw1_h2 = [wsh2.tile([P, H2], BF16, name=f"sw1_{k}_1", tag=f"sw1_{k}_1")
                      for k in range(KD)]
            sw2_h2 = [wsh2.tile([P, H2], BF16, name=f"sw2_{k}_1", tag=f"sw2_{k}_1")
                      for k in range(KD)]
            sw3_sb = [wdn.tile([P, D], BF16, name=f"sw3_{k}", tag=f"sw3_{k}")
                      for k in range(KH)]
            x01 = [xpool.tile([P, NT], BF16, name=f"x_{k}_1", tag=f"x_{k}_1")
                   for k in range(KD)]
            for k in range(KD):
                nc.sync.dma_start(sw1_h2[k][:], sw1T[k * P:(k + 1) * P, H2:])
                nc.sync.dma_start(sw2_h2[k][:], sw2T[k * P:(k + 1) * P, H2:])
            for k in range(KH // 2):
                nc.sync.dma_start(sw3_sb[k][:], sw3T[k * P:(k + 1) * P, :])
            for k in range(KD // 2):
                nc.sync.dma_start(x01[k][:], xT[k * P:(k + 1) * P, NT:CH])
            for k in range(KH // 2, KH):
                nc.sync.dma_start(sw3_sb[k][:], sw3T[k * P:(k + 1) * P, :])
            for k in range(KD // 2, KD):
                nc.sync.dma_start(x01[k][:], xT[k * P:(k + 1) * P, NT:CH])

            sw_h = [[sw1_h1, sw1_h2], [sw2_h1, sw2_h2]]

            for ch in range(T // CH):
                if ch == 0:
                    x_sb = [[x00[k], x01[k]] for k in range(KD)]
                else:
                    x_sb = [[xpool.tile([P, NT], BF16, name=f"x_{k}_{h}",
                                        tag=f"x_{k}_{h}")
                             for h in range(CH // NT)] for k in range(KD)]
                    for k in range(KD):
                        for h in range(CH // NT):
                            nc.sync.dma_start(
                                x_sb[k][h][:],
                                xT[k * P:(k + 1) * P,
                                   ch * CH + h * NT:ch * CH + (h + 1) * NT])
                for sb in range(CH // NT):
                    otok = slice(ch * CH + sb * NT, ch * CH + (sb + 1) * NT)
                    hs = []
                    for m in range(KH):
                        wh, wm = divmod(m, H2 // P)   # which weight half-tile
                        mm = slice(wm * P, (wm + 1) * P)
                        pg = psA.tile([P, NT], F32, name="pg", tag="pg")
                        pu = psA.tile([P, NT], F32, name="pu", tag="pu")
                        for k in range(KD):
                            nc.tensor.matmul(pg[:], sw_h[0][wh][k][:, mm],
                                             x_sb[k][sb][:],
                                             start=(k == 0), stop=(k == KD - 1))
                        for k in range(KD):
                            nc.tensor.matmul(pu[:], sw_h[1][wh][k][:, mm],
                                             x_sb[k][sb][:],
                                             start=(k == 0), stop=(k == KD - 1))
                        sg = tpool.tile([P, NT], F32, name="sg", tag="sg")
                        nc.scalar.activation(sg[:], pg[:], AF.Silu)
                        h = hpool.tile([P, NT], BF16, name=f"h_{m}", tag=f"h_{m}")
                        nc.vector.tensor_mul(h[:], sg[:], pu[:])
                        hs.append(h)
                    last_sb = (ch == T // CH - 1 and sb == CH // NT - 1)
                    for mo in range(KD):
                        orow = slice(mo * P, (mo + 1) * P)
                        if last_sb and mo == KD - 1:
                            # final tile in two half-width groups (separate
                            # PSUM banks) so the very last output DMA is
                            # half-size: shorter post-matmul drain
                            for hf in range(2):
                                cs = slice(hf * (NT // 2), (hf + 1) * (NT // 2))
                                po = psB.tile([P, NT], F32, name="po", tag="po")
                                for k in range(KH):
                                    nc.tensor.matmul(po[:, :NT // 2],
                                                     sw3_sb[k][:, orow],
                                                     hs[k][:, cs],
                                                     start=(k == 0), stop=(k == KH - 1))
                                so = opool.tile([P, NT], BF16, name="so", tag="so")
                                nc.vector.tensor_copy(so[:, :NT // 2], po[:, :NT // 2])
                                nc.sync.dma_start(
                                    shared_outT[orow,
                                                otok.start + hf * (NT // 2):
                                                otok.start + (hf + 1) * (NT // 2)],
                                    so[:, :NT // 2])
                            continue
                        po = psB.tile([P, NT], F32, name="po", tag="po")
                        for k in range(KH):
                            nc.tensor.matmul(po[:], sw3_sb[k][:, orow],
                                             hs[k][:],
                                             start=(k == 0), stop=(k == KH - 1))
                        so = opool.tile([P, NT], BF16, name="so", tag="so")
                        nc.vector.tensor_copy(so[:], po[:])
                        if last_sb:
                            # loads are done; the idle HWDGE drains the tail
                            nc.sync.dma_start(shared_outT[orow, otok], so[:])
                        else:
                            nc.gpsimd.dma_start(shared_outT[orow, otok], so[:])

    nc.compile()
    return nc


_PROGRAM_CACHE: dict = {}


def _get_program(C: int):
    if C not in _PROGRAM_CACHE:
        _PROGRAM_CACHE[C] = _build_program(C)
    return _PROGRAM_CACHE[C]


def _route_like_reference(xf: np.ndarray, router_w: np.ndarray,
                          expert_bias: np.ndarray):
    """Router computed with jax on CPU to bit-match the reference's top-k."""
    import jax
    import jax.numpy as jnp

    cpu = jax.devices("cpu")[0]
    with jax.default_device(cpu):
        xj = jnp.asarray(xf)
        scores = jax.nn.sigmoid(xj @ jnp.asarray(router_w).T)        # (T, E)
        sel = scores + jnp.asarray(expert_bias)
        _, top_idx = jax.lax.top_k(sel, TOPK)                        # (T, K)
        top_sc = jnp.take_along_axis(scores, top_idx, axis=-1)
        top_w = top_sc / (top_sc.sum(-1, keepdims=True) + 1e-9)
        return np.asarray(top_idx), np.asarray(top_w)


def kernel(x, w12, w3, router_w, expert_bias, sw1, sw2, sw3):
    x = np.asarray(x, dtype=np.float32)
    w12 = np.asarray(w12, dtype=np.float32)
    w3 = np.asarray(w3, dtype=np.float32)
    router_w = np.asarray(router_w, dtype=np.float32)
    expert_bias = np.asarray(expert_bias, dtype=np.float32)
    sw1 = np.asarray(sw1, dtype=np.float32)
    sw2 = np.asarray(sw2, dtype=np.float32)
    sw3 = np.asarray(sw3, dtype=np.float32)

    xf = x.reshape(T, D)
    top_idx, top_w = _route_like_reference(xf, router_w, expert_bias)

    # per-expert token lists + combine weights
    idx_list, w_list = [], []
    for e in range(E):
        hit = top_idx == e                      # (T, K)
        tok = np.nonzero(hit.any(axis=1))[0]
        wt = (top_w * hit).sum(axis=1)[tok]     # combine weight per token
        idx_list.append(tok.astype(np.int64))
        w_list.append(wt.astype(np.float32))

    max_n = max(len(i) for i in idx_list)
    # Device capacity policy: cap at C_CORE (the exact mean load for top-2 of
    # 8 experts) and fix up small per-expert overflows on host in fp32
    # (<0.2% of FLOPs, like the router). Grossly imbalanced routing falls
    # back to extra device launches in slabs of C_MAX.
    C_CORE = 1024
    C_MAX = 1280   # slab size for the imbalanced-routing fallback (SBUF limit)
    overflow = sum(max(0, len(i) - C_CORE) for i in idx_list)
    if max_n <= C_CORE:
        C = max(P, -(-max_n // P) * P)          # capacity, multiple of 128
        n_launches, host_fix = 1, False
    elif overflow <= 1024:
        C, n_launches, host_fix = C_CORE, 1, True
    else:
        C = C_MAX
        n_launches, host_fix = max(1, -(-max_n // C_MAX)), False

    xT16 = np.ascontiguousarray(xf.T).astype(ml_dtypes.bfloat16)   # (D, T)

    nc = _get_program(C)

    sw_z = np.zeros((D, HC), dtype=ml_dtypes.bfloat16)
    sw3_z = np.zeros((HC, D), dtype=ml_dtypes.bfloat16)

    outT = np.zeros((D, T), dtype=np.float32)
    global _LAST_RESULTS
    for launch in range(n_launches):
        lo = launch * C_MAX
        in_maps = []
        for c in range(NCORES):
            hs = slice(c * HC, (c + 1) * HC)
            idx_c = idx_list[c][lo:lo + C]
            w_c = w_list[c][lo:lo + C]
            n_c = len(idx_c)
            xg = np.zeros((D, C), dtype=ml_dtypes.bfloat16)
            xg[:, :n_c] = xT16[:, idx_c]
            if launch == 0:
                s1 = np.ascontiguousarray(sw1[hs].T).astype(ml_dtypes.bfloat16)
                s2 = np.ascontiguousarray(sw2[hs].T).astype(ml_dtypes.bfloat16)
                s3 = np.ascontiguousarray(sw3[:, hs].T).astype(ml_dtypes.bfloat16)
            else:
                s1, s2, s3 = sw_z, sw_z, sw3_z   # shared part already done
            # reorder w12 columns into [gate m0-3 | up m0-3 | gate m4-7 | up m4-7]
            w12t = np.ascontiguousarray(w12[c].T).astype(ml_dtypes.bfloat16)
            w12r = np.concatenate([w12t[:, 0:NT], w12t[:, HR:HR + NT],
                                   w12t[:, NT:HR], w12t[:, HR + NT:]], axis=1)
            in_maps.append({
                "xT": xT16,
                "sw1T": s1, "sw2T": s2, "sw3T": s3,
                "w12rT": np.ascontiguousarray(w12r),
                "w3T": np.ascontiguousarray(w3[c].T).astype(ml_dtypes.bfloat16),
                "xgT": xg,
            })

        res = run_bass_kernel_spmd(nc, in_maps, core_ids=list(range(NCORES)),
                                   **_RUN_KWARGS)
        _LAST_RESULTS = res

        for c in range(NCORES):
            if launch == 0:
                outT += res.results[c]["shared_outT"].astype(np.float32)
            idx_c = idx_list[c][lo:lo + C]
            if len(idx_c):
                # token indices are unique within one expert; combine weight
                # applied here in fp32
                ro = res.results[c]["routed_outT"][:, :len(idx_c)].astype(np.float32)
                outT[:, idx_c] += ro * w_list[c][lo:lo + C][None, :]

    if host_fix:
        # fp32 fixup for tokens beyond the device capacity of each expert
        for c in range(NCORES):
            tail = idx_list[c][C:]
            if len(tail) == 0:
                continue
            wts = w_list[c][C:]
            xs = xf[tail]                             # (n, D)
            h12 = xs @ w12[c].T                       # (n, 2*HR)
            h1, h2 = h12[:, :HR], h12[:, HR:]
            h = h1 / (1.0 + np.exp(-h1)) * h2         # silu(h1) * h2
            out = (h * wts[:, None]) @ w3[c].T        # (n, D)
            outT[:, tail] += out.T
    return outT.T.reshape(B, S, D).astype(np.float32)


# test harness hooks: set _RUN_KWARGS = {"trace": True, ...} before calling
# kernel() to profile; read _LAST_RESULTS afterwards.
_RUN_KWARGS: dict = {}
_LAST_RESULTS = None

